# revision 84
# baseline (speedup 1.0000x reference)
"""MemoryAugmentedLayer kernel v4 for 8 trn2 NeuronCores.

Data-parallel over batch B=32768 (4096 rows/core); the einsum partial
sums ride one bf16 AllReduce between the write and read phases.

Design (147us v2 baseline -> ~88us; ACT-engine exp throughput bound):
- fp8e4 DoubleRow matmuls (0.5 cyc/row): kv/vv projections (D=256 as
  2x128 slots), the write einsum, and the dH update. Query projection,
  read logits, u and s matmuls stay bf16 (fp8 there costs 2-4e-2 rel
  err on y; fp8 on the einsum operands costs ~nothing).
- Write-softmax exp batched [128,1024] with accum_out row-sums; no DVE
  op at all on the E_w path. elu = ACT exp + DVE tensor_scalar (4x
  mode) + tensor_tensor (2x mode); scalar_tensor_tensor (1x) avoided.
- The einsum runs E_w-stationary so dK/dV accumulate directly in
  [m, k|v] layout: zero transposes around the AllReduce, and the
  post-AR chain is a handful of DVE ops that never parks the PE queue.
- Pipeline per rep: all 8 front chunks of rep r+1 are emitted before
  rep r's tail; the AR launches right after the einsum (before phase 2
  of r-1 in the PE queue) so it flies under ~60us of ACT work; next-rep
  input DMAs are prefetched before the red DMA so the in-order SP queue
  never parks them behind the collective.
- kv/vv normalize-scale copies write fp8 with a x256 range shift
  (removed post-AR) so fp8 stays in its normal range.
- Biases: bk/bv/bq/brd ride ACT bias ports (free, general); nonzero bwr
  needs a rank-1 PE accumulate, handled by the general build. kernel()
  dispatches on the actual bias values (zb build for the all-zero
  case); both paths verified on hardware at ~7.6e-3 rel err.
"""

import numpy as np

import concourse.bacc as bacc
import concourse.mybir as mybir
import concourse.tile as tile
from concourse import masks
from concourse.bass_utils import run_bass_kernel_spmd

F32 = mybir.dt.float32
BF16 = mybir.dt.bfloat16
FP8 = mybir.dt.float8e4
FP8_NP = mybir.dt.np(FP8)

B, D, M, K, V = 32768, 256, 1024, 128, 128
N_CORES = 8
B_LOC = B // N_CORES          # 4096 rows per core
CHUNK = 512                   # batch columns processed per chunk
NCH = B_LOC // CHUNK          # 8 chunks
NBT = CHUNK // 128            # 4 batch tiles of 128 per chunk
MT = M // 128                 # 8 tiles of the memory dim
DT = D // 128                 # 2 tiles of the input dim
INV_B = 1.0 / B
SCALE_KV = 256.0              # range shift for fp8 kv_s/vv_s
SCALE_AR = 1.0               # range shift for the AllReduce payload
CC_DT = BF16                  # collective payload dtype
F_POST = 8                    # post-AR emitted after all fronts: the AR
                              # gets the whole front window; the short
                              # DVE-only post-AR chain hides in the queue
BENCH_BUILD_KW = {}


def build_nc(repeat=1, zb=True):
    nc = bacc.Bacc("TRN2", target_bir_lowering=False, debug=False,
                   num_devices=N_CORES)

    xT8 = nc.dram_tensor("xT8", [128, DT, B_LOC], FP8, kind="ExternalInput")
    xTb = nc.dram_tensor("xTb", [128, DT, B_LOC], BF16, kind="ExternalInput")
    Wk = nc.dram_tensor("Wk", [D, K], F32, kind="ExternalInput")
    Wv = nc.dram_tensor("Wv", [D, V], F32, kind="ExternalInput")
    Wq = nc.dram_tensor("Wq", [D, K], F32, kind="ExternalInput")
    bk = nc.dram_tensor("bk", [K, 1], F32, kind="ExternalInput")
    bv = nc.dram_tensor("bv", [V, 1], F32, kind="ExternalInput")
    bq = nc.dram_tensor("bq", [K, 1], F32, kind="ExternalInput")
    Wwr = nc.dram_tensor("Wwr", [M, M], F32, kind="ExternalInput")
    Wrd = nc.dram_tensor("Wrd", [M, M], F32, kind="ExternalInput")
    bwr = nc.dram_tensor("bwr", [M, 1], F32, kind="ExternalInput")
    brd = nc.dram_tensor("brd", [M, 1], F32, kind="ExternalInput")
    km = nc.dram_tensor("key_memory", [M, K], F32, kind="ExternalInput")
    vm = nc.dram_tensor("value_memory", [M, V], F32, kind="ExternalInput")
    y = nc.dram_tensor("y", [B_LOC, V], F32, kind="ExternalOutput")

    with tile.TileContext(nc) as tc:
        _emit(nc, tc, xT8, xTb, Wk, Wv, Wq, bk, bv, bq, Wwr, Wrd, bwr, brd,
              km, vm, y, repeat=repeat, zb=zb)
    nc.compile()
    return nc


def _emit(nc, tc, xT8, xTb, Wk, Wv, Wq, bk, bv, bq, Wwr, Wrd, bwr, brd, km,
          vm, y, repeat=1, zb=True):
    AF = mybir.ActivationFunctionType
    ALU = mybir.AluOpType

    with (
        tc.tile_pool(name="resident", bufs=1) as rp,
        tc.tile_pool(name="stage", bufs=2) as stage,
        tc.tile_pool(name="stream", bufs=2) as sp,
        tc.tile_pool(name="ps_mm", bufs=2, space="PSUM") as ps_mm,
        tc.tile_pool(name="ps_pp", bufs=1, space="PSUM") as ps_pp,
        tc.tile_pool(name="ps_tr", bufs=1, space="PSUM") as ps_tr,
        tc.tile_pool(name="ps_u", bufs=1, space="PSUM") as ps_u,
        tc.tile_pool(name="ps_s", bufs=1, space="PSUM") as ps_s,
        tc.tile_pool(name="dram", bufs=1, space="DRAM") as dp,
    ):
        # ---------------- setup ----------------
        ident = rp.tile([128, 128], F32)
        masks.make_identity(nc, ident[:])
        ident_b = rp.tile([128, 128], BF16)
        nc.vector.tensor_copy(ident_b[:], ident[:])
        one1 = rp.tile([1, 1], F32)
        nc.gpsimd.memset(one1[:], 1.0)
        ones_b = rp.tile([128, 1], BF16)
        nc.gpsimd.memset(ones_b[:], 1.0)

        # kv/vv projection weights: DoubleRow stationary fp8; query
        # projection stays bf16 (qv feeds the read logits; fp8 there costs
        # ~2.4e-2 rel err on y)
        projw8 = rp.tile([128, DT, 2, 128], FP8)
        projwq = rp.tile([128, DT, 128], BF16)
        for j, W in enumerate((Wk, Wv, Wq)):
            for dt_ in range(DT):
                wst = stage.tile([128, 128], F32, tag="wst")
                nc.sync.dma_start(wst[:], W[dt_ * 128:(dt_ + 1) * 128, :])
                if j < 2:
                    nc.vector.tensor_copy(projw8[:, dt_, j, :], wst[:])
                else:
                    nc.vector.tensor_copy(projwq[:, dt_, :], wst[:])

        bias_p = rp.tile([128, 3], F32)
        for j, b in enumerate((bk, bv, bq)):
            nc.sync.dma_start(bias_p[:, j:j + 1], b[:])
        bias_pm1 = rp.tile([128, 3], F32)
        nc.vector.tensor_scalar_add(bias_pm1[:], bias_p[:], -1.0)

        bwr_row = ones_r1 = None
        if not zb:
            bwr_row = rp.tile([1, M], F32)
            nc.sync.dma_start(bwr_row[:],
                              bwr.rearrange("(o m) x -> o (m x)", o=1))
            ones_r1 = rp.tile([1, 128], F32)
            nc.gpsimd.memset(ones_r1[:], 1.0)
        bias_rd = rp.tile([128, MT], F32)
        for mp in range(MT):
            nc.sync.dma_start(bias_rd[:, mp:mp + 1],
                              brd[mp * 128:(mp + 1) * 128, :])

        # G = km.T @ Wwr and H0 = km.T @ Wrd (bf16 residents)
        G_b = rp.tile([128, M], BF16)
        H0_b = rp.tile([128, M], BF16)
        g_ps = ps_mm.tile([128, M], F32, tag="mm", name="g_ps")
        h_ps = ps_mm.tile([128, M], F32, tag="mm", name="h_ps")
        for mk in range(MT):
            mst = stage.tile([128, 128], F32, tag="mst")
            nc.sync.dma_start(mst[:], km[mk * 128:(mk + 1) * 128, :])
            km_b = stage.tile([128, 128], BF16, tag="km_b")
            nc.vector.tensor_copy(km_b[:], mst[:])
            wwrt = stage.tile([128, M], F32, tag="wbig")
            nc.sync.dma_start(wwrt[:], Wwr[mk * 128:(mk + 1) * 128, :])
            wwrt_b = stage.tile([128, M], BF16, tag="wbig_b")
            nc.vector.tensor_copy(wwrt_b[:], wwrt[:])
            wrdt = stage.tile([128, M], F32, tag="wbig2")
            nc.sync.dma_start(wrdt[:], Wrd[mk * 128:(mk + 1) * 128, :])
            wrdt_b = stage.tile([128, M], BF16, tag="wbig2_b")
            nc.vector.tensor_copy(wrdt_b[:], wrdt[:])
            for half in range(2):
                nc.tensor.matmul(g_ps[:, half * 512:(half + 1) * 512], km_b[:],
                                 wwrt_b[:, half * 512:(half + 1) * 512],
                                 start=(mk == 0), stop=(mk == MT - 1),
                                 skip_group_check=True)
                nc.tensor.matmul(h_ps[:, half * 512:(half + 1) * 512], km_b[:],
                                 wrdt_b[:, half * 512:(half + 1) * 512],
                                 start=(mk == 0), stop=(mk == MT - 1),
                                 skip_group_check=True)
        nc.vector.tensor_copy(G_b[:], g_ps[:])
        nc.vector.tensor_copy(H0_b[:], h_ps[:])

        # Wrd fp8 DoubleRow moving tiles for dH: [m-part, pair, slot, M]
        wrd8 = rp.tile([128, MT // 2, 2, M], FP8)
        for p in range(MT // 2):
            for i in range(2):
                wrdt2 = stage.tile([128, M], F32, tag="wbig")
                nc.sync.dma_start(
                    wrdt2[:], Wrd[(2 * p + i) * 128:(2 * p + i + 1) * 128, :])
                nc.vector.tensor_copy(wrd8[:, p, i, :], wrdt2[:])

        # value memory bf16 resident [m-part, mk, v]
        vm_b = rp.tile([128, MT, 128], BF16)
        for mk in range(MT):
            vmst = stage.tile([128, 128], F32, tag="mst")
            nc.sync.dma_start(vmst[:], vm[mk * 128:(mk + 1) * 128, :])
            nc.vector.tensor_copy(vm_b[:, mk, :], vmst[:])

        # ---------------- per-rep resident streams ----------------
        qryT_bufs = [rp.tile([128, B_LOC], BF16, name=f"qryT{i}")
                     for i in range(2)]
        E_w = rp.tile([128, NCH, NBT, M], FP8)
        kvv_s = rp.tile([128, NCH, NBT, 256], FP8)

        xT_tiled = xT8.rearrange("p s (h c) -> h p s c", c=CHUNK)
        xTb_tiled = xTb.rearrange("p s (h c) -> h p s c", c=CHUNK)
        y_tiled = y.rearrange("(h t p) v -> h p t v", p=128, t=NBT)

        fargs = (nc, sp, ps_mm, ps_pp, ps_tr, ident_b, projw8, projwq, bias_p,
                 bias_pm1, bwr_row, ones_r1, G_b, E_w, kvv_s, zb)
        margs = (nc, rp, sp, ps_mm, ps_tr, ident_b, wrd8, H0_b, vm_b)
        pargs = (nc, rp, sp, ps_mm, ps_tr, ps_u, ps_pp, ps_s, ident_b, one1,
                 ones_b, bias_rd, y_tiled, zb)

        def prefetch(rep_i):
            tiles = []
            for h in range(NCH):
                xTc = sp.tile([128, DT, CHUNK], FP8, tag="xTc", bufs=10,
                              name=f"xTc{rep_i}_{h}")
                nc.sync.dma_start(xTc[:], xT_tiled[h])
                xTcb = sp.tile([128, DT, CHUNK], BF16, tag="xTcb", bufs=10,
                               name=f"xTcb{rep_i}_{h}")
                nc.sync.dma_start(xTcb[:], xTb_tiled[h])
                tiles.append((xTc, xTcb))
            return tiles

        def fetch_red(cc_out, rep_i):
            # red DMA is emitted AFTER the next rep's input prefetch, so
            # the in-order SP queue never parks input DMAs behind the
            # collective wait
            red = rp.tile([128, 2 * M], CC_DT, tag="red", name=f"red{rep_i}")
            nc.sync.dma_start(red[:], cc_out[:])
            return red

        prev = None
        tiles = prefetch(0)
        for _rep in range(repeat):
            qry = qryT_bufs[_rep % 2]
            for h in range(F_POST):
                _emit_front(*fargs, tiles[h], qry, h)
            mid = None
            if prev is not None:
                red = fetch_red(prev[0], _rep)
                mid = _emit_postar(*margs, red)
            for h in range(F_POST, NCH):
                _emit_front(*fargs, tiles[h], qry, h)
            if _rep + 1 < repeat:
                tiles = prefetch(_rep + 1)
            cc_out = _emit_ar(nc, rp, dp, ps_mm, kvv_s, E_w, _rep)
            if prev is not None:
                _emit_phase2(*pargs, *mid, prev[1])
            prev = (cc_out, qry)
        red = fetch_red(prev[0], repeat)
        mid = _emit_postar(*margs, red)
        _emit_phase2(*pargs, *mid, prev[1])


def _out_dve(nc, sp, st):
    u_ps, s_ps, h = st
    s_sb = sp.tile([1, CHUNK], F32, tag="s_sb", bufs=3, name=f"ssb{h}")
    nc.vector.tensor_copy(s_sb[:], s_ps)
    u_sb = sp.tile([128, CHUNK], BF16, tag="u_sb", bufs=3, name=f"usb{h}")
    nc.vector.tensor_copy(u_sb[:], u_ps[:])
    return (s_sb, u_sb, h)


def _out_pe(nc, sp, ps_mm, ps_tr, ident_b, one1, y_tiled, st):
    s_sb, u_sb, h = st
    pst = ps_mm.tile([128, 2 * CHUNK], F32, tag="mm")
    for t in range(NBT):
        nc.tensor.matmul(pst[:, t:t + 1], s_sb[0:1, t * 128:(t + 1) * 128],
                         one1[:], start=True, stop=True,
                         skip_group_check=True)
    s_colsr = sp.tile([128, NBT], F32, tag="s_colsr", bufs=2)
    nc.vector.tensor_copy(s_colsr[:], pst[:, 0:NBT])
    r_cols = sp.tile([128, NBT], F32, tag="r_cols", bufs=2)
    nc.vector.reciprocal(r_cols[:], s_colsr[:])
    ptu = ps_tr.tile([128, 2 * NBT, 128], BF16, tag="trb")
    ot = sp.tile([128, NBT, V], F32, tag="ot", bufs=2)
    for t in range(NBT):
        nc.tensor.matmul(ptu[:, t, :], u_sb[:, t * 128:(t + 1) * 128],
                         ident_b[:], is_transpose=True,
                         start=True, stop=True, skip_group_check=True)
    for t in range(NBT):
        nc.vector.tensor_scalar_mul(ot[:, t, :], ptu[:, t, :],
                                    r_cols[:, t:t + 1])
    nc.sync.dma_start(y_tiled[h], ot[:])


def _emit_front(nc, sp, ps_mm, ps_pp, ps_tr, ident_b, projw8, projwq, bias_p,
                bias_pm1, bwr_row, ones_r1, G_b, E_w, kvv_s, zb,
                tiles, qryT, h):
    """One chunk of phase 1: projections+elu, write logits + batched exp
    with accum row-sums, transposes + fp8 normalize-scale copies."""
    AF = mybir.ActivationFunctionType
    ALU = mybir.AluOpType

    xTc, xTcb = tiles

    # projections + elu: elu(z+b) = min(exp(z+b),1) + max(z+b-1,-1)
    kvT = sp.tile([128, CHUNK], BF16, tag="kvT", bufs=2)
    vvT = sp.tile([128, CHUNK], BF16, tag="vvT", bufs=2)
    for j in range(3):
        ppt = ps_pp.tile([128, CHUNK], F32, tag="pp")
        pp = ppt[:]
        if j < 2:
            nc.tensor.matmul(pp, projw8[:, :, j, :], xTc[:],
                             start=True, stop=True,
                             perf_mode=mybir.MatmulPerfMode.DoubleRow)
        else:
            for dt_ in range(DT):
                nc.tensor.matmul(pp, projwq[:, dt_, :], xTcb[:, dt_, :],
                                 start=(dt_ == 0), stop=(dt_ == DT - 1))
        texp = sp.tile([128, CHUNK], BF16, tag="texp", bufs=2)
        nc.scalar.activation(texp[:], pp, AF.Exp, bias=bias_p[:, j:j + 1])
        trelu = sp.tile([128, CHUNK], BF16, tag="trelu", bufs=2)
        nc.vector.tensor_scalar(out=trelu[:], in0=pp,
                                scalar1=bias_pm1[:, j:j + 1],
                                scalar2=-1.0, op0=ALU.add, op1=ALU.max)
        tmin = sp.tile([128, CHUNK], BF16, tag="tmin", bufs=2)
        nc.vector.tensor_scalar_min(tmin[:], texp[:], 1.0)
        dst = (kvT[:], vvT[:], qryT[:, h * CHUNK:(h + 1) * CHUNK])[j]
        nc.vector.tensor_tensor(dst, tmin[:], trelu[:], ALU.add)

    # write logits (batched 1024-wide) + exp + row sums
    s_cols = sp.tile([128, NBT], F32, tag="s_cols", bufs=2)
    for t in range(NBT):
        zt = ps_mm.tile([128, M], F32, tag="mm")
        kvblk = kvT[:, t * 128:(t + 1) * 128]
        for half in range(2):
            nc.tensor.matmul(zt[:, half * 512:(half + 1) * 512], kvblk,
                             G_b[:, half * 512:(half + 1) * 512],
                             start=True, stop=(bwr_row is None),
                             skip_group_check=True)
            if bwr_row is not None:
                nc.tensor.matmul(zt[:, half * 512:(half + 1) * 512],
                                 ones_r1[:],
                                 bwr_row[:, half * 512:(half + 1) * 512],
                                 start=False, stop=True, skip_group_check=True)
        nc.scalar.activation(E_w[:, h, t, :], zt[:], AF.Exp,
                             accum_out=s_cols[:, t:t + 1])

    rw = sp.tile([128, NBT], F32, tag="rw", bufs=2)
    nc.vector.reciprocal(rw[:], s_cols[:])
    rw256 = sp.tile([128, NBT], F32, tag="rw256", bufs=2)
    nc.vector.tensor_scalar_mul(rw256[:], rw[:], SCALE_KV)

    ptkv = ps_tr.tile([128, 2 * NBT, 128], BF16, tag="trb")
    for t in range(NBT):
        nc.tensor.matmul(ptkv[:, t, :], kvT[:, t * 128:(t + 1) * 128],
                         ident_b[:], is_transpose=True,
                         start=True, stop=True, skip_group_check=True)
    for t in range(NBT):
        nc.tensor.matmul(ptkv[:, NBT + t, :], vvT[:, t * 128:(t + 1) * 128],
                         ident_b[:], is_transpose=True,
                         start=True, stop=True, skip_group_check=True)
    for t in range(NBT):
        nc.vector.tensor_scalar_mul(kvv_s[:, h, t, 0:128], ptkv[:, t, :],
                                    rw256[:, t:t + 1])
        nc.vector.tensor_scalar_mul(kvv_s[:, h, t, 128:256],
                                    ptkv[:, NBT + t, :], rw256[:, t:t + 1])
    if _DBG_CB is not None:
        _DBG_CB(nc, sp, h, locals())


_DBG_CB = None


def _emit_ar(nc, rp, dp, ps_mm, kvv_s, E_w, rep_i):
    """Einsum with E_w stationary (fp8 DoubleRow): accumulates directly in
    [m, k|v] layout -- no transposes on either side of the AllReduce, so
    the post-AR chain is a handful of DVE ops and never parks the PE."""
    part = rp.tile([128, 2 * M], CC_DT, tag="part", name=f"part{rep_i}")
    accs = [ps_mm.tile([128, M], F32, tag="mm", name=f"acc{i}_{rep_i}")
            for i in range(2)]
    for h in range(NCH):
        for tp in range(NBT // 2):
            for mt in range(MT):
                acc = accs[mt // 4]
                nc.tensor.matmul(
                    acc[:, (mt % 4) * 256:(mt % 4) * 256 + 256],
                    E_w[:, h, 2 * tp:2 * tp + 2, mt * 128:(mt + 1) * 128],
                    kvv_s[:, h, 2 * tp:2 * tp + 2, :],
                    start=(h == 0 and tp == 0),
                    stop=(h == NCH - 1 and tp == 1),
                    skip_group_check=True,
                    perf_mode=mybir.MatmulPerfMode.DoubleRow)
    for i in range(2):
        nc.vector.tensor_scalar_mul(part[:, i * M:(i + 1) * M], accs[i][:],
                                    1.0 / SCALE_AR)
    cc_in = dp.tile([128, 2 * M], CC_DT, tag="cc_in", name=f"cci{rep_i}")
    cc_out = dp.tile([128, 2 * M], CC_DT, tag="cc_out", name=f"cco{rep_i}")
    nc.sync.dma_start(cc_in[:], part[:])
    nc.gpsimd.collective_compute(
        "AllReduce", mybir.AluOpType.add,
        replica_groups=[list(range(N_CORES))],
        ins=[cc_in.opt()], outs=[cc_out.opt()],
    )
    return cc_out


def _emit_postar(nc, rp, sp, ps_mm, ps_tr, ident_b, wrd8, H0_b, vm_b, red):
    """Post-AR: rescale dK to fp8, apply dV to the value memory, dH + H.
    red is [m-part, (mt, k|v)]: red[:, mt*256 : mt*256+128] is dK's m-tile
    mt, the next 128 columns are dV's."""
    ALU = mybir.AluOpType
    rkm8 = rp.tile([128, MT, 128], FP8, tag="rkm8")
    vmn_b = rp.tile([128, MT, 128], BF16, tag="vmn_b")
    for mk in range(MT):
        nc.vector.tensor_scalar_mul(rkm8[:, mk, :],
                                    red[:, mk * 256:mk * 256 + 128],
                                    SCALE_AR / SCALE_KV)
        nc.vector.scalar_tensor_tensor(vmn_b[:, mk, :],
                                       red[:, mk * 256 + 128:mk * 256 + 256],
                                       SCALE_AR * INV_B / SCALE_KV,
                                       vm_b[:, mk, :], ALU.mult, ALU.add)
    dh = ps_mm.tile([128, M], F32, tag="mm", name="dh")
    for half in range(2):
        for p in range(MT // 2):
            nc.tensor.matmul(dh[:, half * 512:(half + 1) * 512],
                             rkm8[:, 2 * p:2 * p + 2, :],
                             wrd8[:, p, :, half * 512:(half + 1) * 512],
                             start=(p == 0), stop=(p == MT // 2 - 1),
                             skip_group_check=True,
                             perf_mode=mybir.MatmulPerfMode.DoubleRow)
    H_b = rp.tile([128, M], BF16, tag="H_b")
    nc.vector.scalar_tensor_tensor(H_b[:], dh[:], INV_B, H0_b[:],
                                   ALU.mult, ALU.add)
    return H_b, vmn_b


def _emit_phase2(nc, rp, sp, ps_mm, ps_tr, ps_u, ps_pp, ps_s, ident_b, one1,
                 ones_b, bias_rd, y_tiled, zb, H_b, vmn_b, qryT):
    AF = mybir.ActivationFunctionType

    # ---- phase 2: read softmax + value gather ----
    # software-pipelined: the u/s consume of pair p is emitted AFTER pair
    # p+1's prl matmuls, so the in-order PE queue never blocks the ACT
    # exp chain (PE stalls on erT(p) would otherwise delay prl(p+1)).
    def consume(st):
        u_ps, s_ps, erT, p, _h = st
        for i in range(2):
            mp = 2 * p + i
            nc.tensor.matmul(u_ps[:], vmn_b[:, mp, :], erT[:, i, :],
                             start=(mp == 0), stop=(mp == MT - 1),
                             skip_group_check=True)
            nc.tensor.matmul(s_ps, ones_b[:], erT[:, i, :],
                             start=(mp == 0), stop=(mp == MT - 1),
                             skip_group_check=True)

    # rows 0 and 64: matmul out base partition must be 0/32/64
    s2 = ps_s.tile([128, CHUNK], F32, tag="s", name="s2")
    pend = None        # (u_ps, s_ps, erT, p, h)
    out_dve = None     # (u_ps, s_ps, h): accumulation done, copies pending
    out_pe = None      # (s_sb, u_sb, h): copies emitted, PE stage pending
    for h in range(NCH):
        qslice = qryT[:, h * CHUNK:(h + 1) * CHUNK]
        u_ps = ps_u.tile([128, CHUNK], F32, tag="u", name=f"u{h}")
        srow = 64 * (h % 2)
        s_ps = s2[srow:srow + 1, :]
        for p in range(MT // 2):
            prl = ps_mm.tile([128, 2 * CHUNK], F32, tag="mm")
            erT = sp.tile([128, 2, CHUNK], BF16, tag="erT", bufs=4)
            for i in range(2):
                nc.tensor.matmul(prl[:, i * CHUNK:(i + 1) * CHUNK],
                                 H_b[:, (2 * p + i) * 128:(2 * p + i + 1) * 128],
                                 qslice, start=True, stop=True,
                                 skip_group_check=True)
            if zb:
                nc.scalar.activation(erT[:], prl[:], AF.Exp)
            else:
                for i in range(2):
                    nc.scalar.activation(
                        erT[:, i, :], prl[:, i * CHUNK:(i + 1) * CHUNK],
                        AF.Exp, bias=bias_rd[:, 2 * p + i:2 * p + i + 1])
            if out_pe is not None:
                _out_pe(nc, sp, ps_mm, ps_tr, ident_b, one1, y_tiled, out_pe)
                out_pe = None
            if pend is not None:
                consume(pend)
                if pend[3] == MT // 2 - 1:
                    out_dve = (pend[0], pend[1], pend[4])
            if out_dve is not None:
                out_pe = _out_dve(nc, sp, out_dve)
                out_dve = None
            pend = (u_ps, s_ps, erT, p, h)
    consume(pend)
    st = _out_dve(nc, sp, (pend[0], pend[1], pend[4]))
    if out_pe is not None:
        _out_pe(nc, sp, ps_mm, ps_tr, ident_b, one1, y_tiled, out_pe)
    _out_pe(nc, sp, ps_mm, ps_tr, ident_b, one1, y_tiled, st)


_NC_CACHE = {}


def _get_nc(zb=True):
    if zb not in _NC_CACHE:
        _NC_CACHE[zb] = build_nc(zb=zb)
    return _NC_CACHE[zb]


def make_in_maps(inputs):
    xs = np.ascontiguousarray(np.asarray(inputs["x"], dtype=np.float32))
    rep = {}
    for name in ("Wk", "Wv", "Wq", "Wwr", "Wrd", "key_memory", "value_memory"):
        rep[name] = np.ascontiguousarray(np.asarray(inputs[name], np.float32))
    for name in ("bk", "bv", "bq", "bwr", "brd"):
        rep[name] = np.ascontiguousarray(
            np.asarray(inputs[name], np.float32).reshape(-1, 1))
    in_maps = []
    import ml_dtypes
    for c in range(N_CORES):
        xc = xs[c * B_LOC:(c + 1) * B_LOC]            # [B_LOC, D]
        xT = np.ascontiguousarray(xc.T)               # [D, B_LOC]
        xr = np.ascontiguousarray(
            xT.reshape(DT, 128, B_LOC).transpose(1, 0, 2))  # [128, DT, B_LOC]
        m = {"xT8": xr.astype(FP8_NP), "xTb": xr.astype(ml_dtypes.bfloat16)}
        m.update(rep)
        in_maps.append(m)
    return in_maps


def kernel(**inputs):
    zb = not (np.any(np.asarray(inputs["bwr"])) or
              np.any(np.asarray(inputs["brd"])))
    nc = _get_nc(zb=zb)
    in_maps = make_in_maps(inputs)
    res = run_bass_kernel_spmd(nc, in_maps, core_ids=list(range(N_CORES)))
    return np.concatenate([r["y"] for r in res.results], axis=0)
